# revision 3
# baseline (speedup 1.0000x reference)
"""CLIP cross-attention (pre-LN QKV + softmax attention + bottleneck adapter)
on 8 Trainium2 NeuronCores, batch-data-parallel (1 batch element per core).

Per-core dataflow (all shapes per batch element, S=1024 tokens, H=1024):
  LN(hs), LN(ctx) in natural layout (bn_stats), gamma/beta folded into the
  projection weights on the host; PE-transpose the normalized activations to
  [H, S] layout (fp32r); QKV projections in fp32r (full PE rate at N=512,
  ~tf32 precision); scores computed transposed per head S^T = K^T.T @ Q^T
  (fp32r); exp on ScalarE (unnormalized softmax - no max subtraction needed,
  |scores| <= ~9); P^T in bf16; PV as natural-layout matmul with an appended
  ones column for the softmax row-sums; normalize via reciprocal+scale;
  attention output staged through DRAM scratch (SBUF pressure), re-loaded for
  the adapter: D^T = Wd.T @ attn^T, tanh-gelu, U = G^T.T @ [Wu;bu], residual
  add, store.
"""

import numpy as np
import ml_dtypes

import concourse.bass as bass
import concourse.tile as tile
from concourse import bacc, mybir
from concourse.bass_utils import run_bass_kernel_spmd
from concourse.masks import make_identity
from contextlib import ExitStack

F32 = mybir.dt.float32
F32R = mybir.dt.float32r
BF16 = mybir.dt.bfloat16
AF = mybir.ActivationFunctionType
ALU = mybir.AluOpType

S = 1024
H = 1024
NH = 16
HD = 64
P = 128
NCORES = 8
EPS = 1e-5


def build_program():
    nc = bacc.Bacc("TRN2", target_bir_lowering=False, debug=False,
                   num_devices=NCORES)

    hs = nc.dram_tensor("hs", [S, H], F32R, kind="ExternalInput")
    cx = nc.dram_tensor("cx", [S, H], F32R, kind="ExternalInput")
    wq = nc.dram_tensor("wq", [H, H], F32R, kind="ExternalInput")
    wk = nc.dram_tensor("wk", [H, H], F32R, kind="ExternalInput")
    wv = nc.dram_tensor("wv", [H, H], F32R, kind="ExternalInput")
    bq = nc.dram_tensor("bq", [P, 8], F32, kind="ExternalInput")
    bk = nc.dram_tensor("bk", [P, 8], F32, kind="ExternalInput")
    bv = nc.dram_tensor("bv", [1, H], BF16, kind="ExternalInput")
    wd = nc.dram_tensor("wd", [H, HD], BF16, kind="ExternalInput")
    bd = nc.dram_tensor("bd", [HD, 1], F32, kind="ExternalInput")
    wub = nc.dram_tensor("wub", [HD + 1, H], BF16, kind="ExternalInput")
    out = nc.dram_tensor("out", [S, H], F32, kind="ExternalOutput")

    with tile.TileContext(nc) as tc, ExitStack() as ctx:
        pc = ctx.enter_context(tc.tile_pool(name="const", bufs=1))
        pbig = ctx.enter_context(tc.tile_pool(name="big", bufs=2))
        pw = ctx.enter_context(tc.tile_pool(name="w", bufs=8))
        pq = ctx.enter_context(tc.tile_pool(name="q", bufs=1))
        pk = ctx.enter_context(tc.tile_pool(name="k", bufs=1))
        pv = ctx.enter_context(tc.tile_pool(name="v", bufs=1))
        pxl = ctx.enter_context(tc.tile_pool(name="xl", bufs=4))
        pstat = ctx.enter_context(tc.tile_pool(name="stat", bufs=2))
        phst = ctx.enter_context(tc.tile_pool(name="hstrip", bufs=1))
        pout = ctx.enter_context(tc.tile_pool(name="outp", bufs=2))
        pg = ctx.enter_context(tc.tile_pool(name="g", bufs=1))
        pdram = ctx.enter_context(tc.tile_pool(name="dram", bufs=1, space="DRAM"))
        pps_t = ctx.enter_context(tc.tile_pool(name="pst", bufs=2, space="PSUM"))
        pps_m = ctx.enter_context(tc.tile_pool(name="psm", bufs=3, space="PSUM"))
        pps_o = ctx.enter_context(tc.tile_pool(name="pso", bufs=3, space="PSUM"))

        # constants / small inputs
        id0 = pc.tile([P, P], F32)
        make_identity(nc, id0[:])
        idr = pc.tile([P, P], F32R)
        nc.vector.tensor_copy(idr[:], id0[:])
        bq_sb = pc.tile([P, 8], F32)
        nc.sync.dma_start(bq_sb[:], bq[:])
        bk_sb = pc.tile([P, 8], F32)
        nc.sync.dma_start(bk_sb[:], bk[:])
        bv_sb = pc.tile([P, H], BF16)
        nc.sync.dma_start(bv_sb[:], bv[:].partition_broadcast(P)[:, 0, :])
        wd_sb = pc.tile([P, 8, HD], BF16)
        nc.sync.dma_start(wd_sb[:], wd[:].rearrange("(c p) a -> p c a", p=P))
        bd_sb = pc.tile([HD, 1], F32)
        nc.sync.dma_start(bd_sb[:], bd[:])
        wub_sb = pc.tile([HD + 1, H], BF16)
        nc.sync.dma_start(wub_sb[:], wub[:])

        eps_sb = pc.tile([P, 1], F32)
        nc.vector.memset(eps_sb[:], EPS)

        vt = pv.tile([P, 8, NH, HD + 1], BF16, tag="V")
        nc.vector.memset(vt[:, :, :, HD:HD + 1], 1.0)
        gt = pg.tile([HD + 1, H], BF16, tag="gt")
        nc.vector.memset(gt[HD:HD + 1, :], 1.0)

        attn_scr = pdram.tile([S, H], F32R, tag="scr")

        qT = pq.tile([P, 8, S], F32R, tag="qT")
        kT = pk.tile([P, 8, S], F32R, tag="kT")

        def load_w(wdram):
            tiles = []
            for kk in range(8):
                wt = pw.tile([P, H], F32R, tag="wc")
                nc.sync.dma_start(wt[:], wdram[kk * P:(kk + 1) * P, :])
                tiles.append(wt)
            return tiles

        def ln_transpose(xdram, dstT):
            # LN in natural layout (stats over free dim), then PE-transpose
            # each 128x128 block into dstT ([H-part chunk, token] layout).
            for m in range(8):
                xt = pxl.tile([P, H], F32R, tag="xl")
                nc.sync.dma_start(xt[:], xdram[m * P:(m + 1) * P, :])
                x32 = xt[:].bitcast(F32)
                st = pstat.tile([P, 2, 6], F32, tag="st")
                nc.vector.bn_stats(st[:, 0, :], x32[:, 0:512])
                nc.vector.bn_stats(st[:, 1, :], x32[:, 512:1024])
                mv = pstat.tile([P, 2], F32, tag="mv")
                nc.vector.bn_aggr(mv[:], st[:])
                sd = pstat.tile([P, 1], F32, tag="sd")
                nc.scalar.activation(sd[:], mv[:, 1:2], AF.Sqrt, bias=eps_sb[:])
                rstd = pstat.tile([P, 1], F32, tag="rs")
                nc.vector.reciprocal(rstd[:], sd[:])
                nc.vector.tensor_scalar(xt[:], x32, mv[:, 0:1], rstd[:],
                                        ALU.subtract, ALU.mult)
                for hc in range(8):
                    pt = pps_t.tile([P, P], F32R, tag="pt")
                    nc.tensor.transpose(pt[:], xt[:, hc * P:(hc + 1) * P], idr[:])
                    nc.vector.tensor_copy(dstT[:, hc, m * P:(m + 1) * P], pt[:])

        def proj_T(wtiles, srcT, dstT, bias_sb):
            # dstT[:, m8, :] = (W.T @ src^T)[m8 chunk] + bias  (all fp32r)
            for m8 in range(8):
                for n2 in range(2):
                    pm = pps_m.tile([P, 512], F32, tag="pm")
                    for kk in range(8):
                        nc.tensor.matmul(
                            pm[:], wtiles[kk][:, m8 * P:(m8 + 1) * P],
                            srcT[:, kk, n2 * 512:(n2 + 1) * 512],
                            start=(kk == 0), stop=(kk == 7))
                    nc.vector.tensor_scalar(
                        dstT[:, m8, n2 * 512:(n2 + 1) * 512], pm[:],
                        bias_sb[:, m8:m8 + 1], None, ALU.add)

        # ---- phase 1: hs LN + transpose; Q projection
        hsT = pbig.tile([P, 8, S], F32R, tag="big")
        wq_t = load_w(wq)
        ln_transpose(hs, hsT)
        proj_T(wq_t, hsT, qT, bq_sb)

        # ---- phase 2: ctx LN + transpose; K, V projections
        ctxT = pbig.tile([P, 8, S], F32R, tag="big")
        wk_t = load_w(wk)
        ln_transpose(cx, ctxT)
        proj_T(wk_t, ctxT, kT, bk_sb)

        wv_t = load_w(wv)
        for c in range(8):
            for n2 in range(2):
                pm = pps_m.tile([P, 512], F32, tag="pm")
                for kk in range(8):
                    nc.tensor.matmul(
                        pm[:], ctxT[:, kk, c * P:(c + 1) * P],
                        wv_t[kk][:, n2 * 512:(n2 + 1) * 512],
                        start=(kk == 0), stop=(kk == 7))
                h0 = n2 * 8
                nc.vector.tensor_tensor(
                    vt[:, c, h0:h0 + 8, 0:HD], pm[:],
                    bv_sb[:, n2 * 512:(n2 + 1) * 512].rearrange(
                        "p (h c) -> p h c", c=HD),
                    ALU.add)

        # ---- phase 3: attention per head
        for h in range(NH):
            r0 = (h % 2) * HD
            hc = h // 2
            pT = pbig.tile([P, 8, S], BF16, tag="big")
            for c in range(8):
                for n2 in range(2):
                    pm = pps_m.tile([P, 512], F32, tag="pm")
                    nc.tensor.matmul(
                        pm[:], kT[r0:r0 + HD, hc, c * P:(c + 1) * P],
                        qT[r0:r0 + HD, hc, n2 * 512:(n2 + 1) * 512],
                        start=True, stop=True)
                    nc.scalar.activation(pT[:, c, n2 * 512:(n2 + 1) * 512],
                                         pm[:], AF.Exp, scale=0.125)
            hst = phst.tile([P, 8, HD], F32R, tag="hst")
            for m in range(8):
                po = pps_o.tile([P, HD + 1], F32, tag="po")
                for c in range(8):
                    nc.tensor.matmul(po[:], pT[:, c, m * P:(m + 1) * P],
                                     vt[:, c, h, :],
                                     start=(c == 0), stop=(c == 7))
                rs = pstat.tile([P, 1], F32, tag="rs2")
                nc.vector.reciprocal(rs[:], po[:, HD:HD + 1])
                nc.vector.tensor_scalar(hst[:, m, :], po[:, 0:HD], rs[:],
                                        None, ALU.mult)
            nc.sync.dma_start(
                attn_scr[:, h * HD:(h + 1) * HD].rearrange(
                    "(m p) c -> p m c", p=P), hst[:])

        # ---- phase 4: adapter + residual
        attn_T = pbig.tile([P, 8, S], BF16, tag="big")
        for n2 in range(2):
            rets = []
            for mi in range(4):
                m = n2 * 4 + mi
                rt = pxl.tile([P, H], F32R, tag="xl")
                nc.sync.dma_start(rt[:], attn_scr[m * P:(m + 1) * P, :])
                rets.append((m, rt))
            for m, rt in rets:
                for hc2 in range(8):
                    pt = pps_t.tile([P, P], F32R, tag="pt")
                    nc.tensor.transpose(pt[:], rt[:, hc2 * P:(hc2 + 1) * P],
                                        idr[:])
                    nc.vector.tensor_copy(attn_T[:, hc2, m * P:(m + 1) * P],
                                          pt[:])
            pd = pps_m.tile([P, 512], F32, tag="pm")
            for kk in range(8):
                nc.tensor.matmul(pd[0:HD, :], wd_sb[:, kk, :],
                                 attn_T[:, kk, n2 * 512:(n2 + 1) * 512],
                                 start=(kk == 0), stop=(kk == 7))
            nc.scalar.activation(gt[0:HD, n2 * 512:(n2 + 1) * 512], pd[0:HD, :],
                                 AF.Gelu_apprx_tanh, bias=bd_sb[:])
            for m, rt in rets:
                for nH in range(2):
                    pu = pps_m.tile([P, 512], F32, tag="pm")
                    nc.tensor.matmul(pu[:], gt[:, m * P:(m + 1) * P],
                                     wub_sb[:, nH * 512:(nH + 1) * 512],
                                     start=True, stop=True)
                    ot = pout.tile([P, 512], F32, tag="out")
                    nc.vector.tensor_tensor(
                        ot[:], pu[:],
                        rt[:, nH * 512:(nH + 1) * 512].bitcast(F32), ALU.add)
                    nc.sync.dma_start(
                        out[m * P:(m + 1) * P, nH * 512:(nH + 1) * 512], ot[:])

    nc.compile()
    return nc


def make_in_maps(hidden_states, context, Wq, bq, Wk, bk, Wv, bv,
                 q_gamma, q_beta, c_gamma, c_beta, Wd, bd, Wu, bu):
    f32 = np.float32
    # fold LN gamma/beta into the projection weights (host-side)
    wq_e = (q_gamma[:, None] * Wq).astype(f32)
    bq_e = (bq + q_beta @ Wq).astype(f32)
    wk_e = (c_gamma[:, None] * Wk).astype(f32)
    bk_e = (bk + c_beta @ Wk).astype(f32)
    wv_e = (c_gamma[:, None] * Wv).astype(f32)
    bv_e = (bv + c_beta @ Wv).astype(f32)

    bq_r = np.ascontiguousarray(bq_e.reshape(8, P).T)   # [P, 8]
    bk_r = np.ascontiguousarray(bk_e.reshape(8, P).T)
    bv_r = bv_e.reshape(1, H).astype(ml_dtypes.bfloat16)
    wd_b = Wd.astype(ml_dtypes.bfloat16)
    bd_r = bd.reshape(HD, 1).astype(f32)
    wub = np.vstack([Wu, bu.reshape(1, H)]).astype(ml_dtypes.bfloat16)

    shared = {
        "wq": np.ascontiguousarray(wq_e), "wk": np.ascontiguousarray(wk_e),
        "wv": np.ascontiguousarray(wv_e),
        "bq": bq_r, "bk": bk_r, "bv": bv_r,
        "wd": wd_b, "bd": bd_r, "wub": wub,
    }
    in_maps = []
    for b_ in range(NCORES):
        m = dict(shared)
        m["hs"] = np.ascontiguousarray(hidden_states[b_]).astype(f32)
        m["cx"] = np.ascontiguousarray(context[b_]).astype(f32)
        in_maps.append(m)
    return in_maps


_CACHE = {}


def get_program():
    if "nc" not in _CACHE:
        _CACHE["nc"] = build_program()
    return _CACHE["nc"]


def kernel(**inputs):
    nc = get_program()
    in_maps = make_in_maps(**{k: np.asarray(v) for k, v in inputs.items()})
    res = run_bass_kernel_spmd(nc, in_maps, list(range(NCORES)))
    out = np.stack([res.results[c]["out"] for c in range(NCORES)], axis=0)
    return out.astype(np.float32)


if __name__ == "__main__":
    rng = np.random.default_rng(0)
    ins = {
        "hidden_states": rng.standard_normal((8, S, H), dtype=np.float32),
        "context": rng.standard_normal((8, S, H), dtype=np.float32),
        "Wq": rng.standard_normal((H, H), dtype=np.float32) / 32,
        "bq": np.zeros(H, np.float32),
        "Wk": rng.standard_normal((H, H), dtype=np.float32) / 32,
        "bk": np.zeros(H, np.float32),
        "Wv": rng.standard_normal((H, H), dtype=np.float32) / 32,
        "bv": np.zeros(H, np.float32),
        "q_gamma": np.ones(H, np.float32), "q_beta": np.zeros(H, np.float32),
        "c_gamma": np.ones(H, np.float32), "c_beta": np.zeros(H, np.float32),
        "Wd": rng.standard_normal((H, HD), dtype=np.float32) / 32,
        "bd": np.zeros(HD, np.float32),
        "Wu": rng.standard_normal((HD, H), dtype=np.float32) / 8,
        "bu": np.zeros(H, np.float32),
    }
    o = kernel(**ins)
    print("kernel out", o.shape, o.dtype, float(np.abs(o).mean()))
